# revision 2
# baseline (speedup 1.0000x reference)
"""Trainium2 Bass kernel for Qwen-style GQA attention block (B=2,S=2048,H=16,KV=8,D=128).

Sharding (8 cores): batch(2) x si-stripes(2) x head-half(2).
  core c: b=c>>2, sh=(c>>1)&1, hh=c&1
  - Q proj + attention for 8 q-heads (hh half) on 8 causally-balanced si blocks (sh stripes)
  - K/V proj for 4 kv heads over full S (replicated across the 2 stripe cores)
  - pair AllGather of ctx^T between the two head-half cores, then column-split o_proj.
All matmuls bf16 with fp32 PSUM accumulation. Softmax without max-subtraction
(scores are O(1) after QK RMSNorm); denominator via an appended ones-column on V.
"""
import sys

sys.path.insert(0, '/opt/trn_rl_repo')

import numpy as np

import concourse.bass as bass
import concourse.tile as tile
from concourse import mybir
from concourse.vector_clock import ScopedClock, VectorClock

B, S, HID = 2, 2048, 2048
H, KV, D = 16, 8, 128
EPS = 1e-6
SCALE = D ** -0.5
NBLK = S // 128  # 16
# causally balanced si-block stripes: sum(i+1) = 68 for both
MYBLKS = [[0, 2, 4, 6, 9, 11, 13, 15], [1, 3, 5, 7, 8, 10, 12, 14]]

F32 = mybir.dt.float32
BF16 = mybir.dt.bfloat16
AF = mybir.ActivationFunctionType


# ---------------------------------------------------------------------------
# Workarounds: this walrus supports only ONE sync-wait per instruction.
def _patched_drain_and_barrier(self, tick_clock, wait_clock):
    gc = tick_clock.global_clock
    vec = list(gc)
    nz = [i for i, v in enumerate(vec) if v > 0] or [0]
    for i in nz:
        cvec = [vec[j] if j == i else 0 for j in range(len(vec))]
        inst = self.nc.sync.drain()
        wait_clock.add_sem_waits(inst.ins, ScopedClock({None: VectorClock(cvec)}))
    self.nc.all_engine_barrier()
    assert self.sems is not None
    popped = self.nc._tile_sem_poison_stack.pop()
    assert popped is self._sem_poison
    self.nc.clear_and_free_semaphores(list(self.sems.allocated().values()))
    self.nc.all_engine_barrier()


tile.TileContext._drain_and_barrier = _patched_drain_and_barrier


def split_multi_waits(nc):
    for fn in nc.m.functions:
        for blk in fn.blocks:
            insts = list(blk.instructions)
            out = []
            changed = False
            for inst in insts:
                si = inst.sync_info
                if si is not None and len(si.on_wait) > 1:
                    waits = list(si.on_wait)
                    for k, w in enumerate(waits[:-1]):
                        out.append(mybir.InstNoOp(
                            name=f"{inst.name}.w{k}", engine=inst.engine,
                            sync_info=mybir.SyncInfo(on_wait=[w], on_update=[]),
                            text_hint="waitsplit"))
                    si.on_wait = [waits[-1]]
                    changed = True
                out.append(inst)
            if changed:
                blk.instructions[:] = out


# ---------------------------------------------------------------------------
def build_kernel():
    nc = bass.Bass(trn_type='TRN2')
    hT = nc.dram_tensor('hT', [HID, S], F32, kind='ExternalInput')
    qwT = nc.dram_tensor('qwT', [HID, 1024], F32, kind='ExternalInput')
    kwT = nc.dram_tensor('kwT', [HID, 512], F32, kind='ExternalInput')
    vwT = nc.dram_tensor('vwT', [HID, 512], F32, kind='ExternalInput')
    owT = nc.dram_tensor('owT', [2048, 1024], F32, kind='ExternalInput')
    # host-fused rope tables (cos/sin x norm-weight halves), [rows, 4, 64]
    qtab = nc.dram_tensor('qtab', [1024, 4, 64], F32, kind='ExternalInput')
    ktab = nc.dram_tensor('ktab', [S, 4, 64], F32, kind='ExternalInput')
    tri = nc.dram_tensor('tri', [128, 128], F32, kind='ExternalInput')
    iden = nc.dram_tensor('iden', [128, 128], F32, kind='ExternalInput')
    out_e = nc.dram_tensor('out', [1024, 1024], F32, kind='ExternalOutput')

    from contextlib import ExitStack
    with ExitStack() as ctx:
        tc = ctx.enter_context(tile.TileContext(nc))
        pool = lambda name, bufs, **kw: ctx.enter_context(
            tc.tile_pool(name=name, bufs=bufs, **kw))
        p_wq = pool('wq', 16)
        p_wk = pool('wk', 16)
        p_wv = pool('wv', 16)
        p_ht = pool('ht', 16)
        p_qt = pool('qt', 8)
        p_kt = pool('kt', 4)
        p_va = pool('va', 4)
        p_ctm = pool('ctm', 8)
        p_c = pool('const', 1)
        p_w = pool('work', 2)
        p_s = pool('small', 4)
        p_scl = pool('scl', 1)
        p_exp = pool('expb', 4)
        p_out = pool('outb', 2)
        ps_a = pool('psA', 2, space='PSUM')
        ps_s = pool('psS', 2, space='PSUM')
        ps_c = pool('psC', 2, space='PSUM')
        ps_t = pool('psT', 2, space='PSUM')
        p_d = pool('dram', 1, space='DRAM')
        if True:
            # ---- constants / weights (cast to bf16 on load) ----
            tri_s = p_c.tile([128, 128], BF16)
            nc.gpsimd.dma_start(tri_s[:], tri[:])
            iden_s = p_c.tile([128, 128], BF16)
            nc.gpsimd.dma_start(iden_s[:], iden[:])
            qtab_s = p_c.tile([128, 8, 4, 64], BF16)
            nc.gpsimd.dma_start(qtab_s[:], qtab.rearrange('(n p) t d -> p n t d', p=128))
            ktab_s = p_c.tile([128, 16, 4, 64], BF16)
            nc.gpsimd.dma_start(ktab_s[:], ktab.rearrange('(n p) t d -> p n t d', p=128))

            wq_s = [p_wq.tile([128, 1024], BF16, tag='wq', name='wq') for _ in range(16)]
            wk_s = [p_wk.tile([128, 512], BF16, tag='wk', name='wk') for _ in range(16)]
            wv_s = [p_wv.tile([128, 512], BF16, tag='wv', name='wv') for _ in range(16)]
            for ch in range(16):
                r = bass.ts(ch, 128)
                nc.gpsimd.dma_start(wq_s[ch][:], qwT[r, :])
                nc.gpsimd.dma_start(wk_s[ch][:], kwT[r, :])
                nc.gpsimd.dma_start(wv_s[ch][:], vwT[r, :])

            # persistent activation tiles
            QT = [p_qt.tile([128, 1024], BF16, tag='qt', name='qtl') for _ in range(8)]
            KT = [p_kt.tile([128, 2048], BF16, tag='kt', name='ktl') for _ in range(4)]
            VA = [p_va.tile([128, 16, 132], BF16, tag='va', name='va') for _ in range(4)]
            sclK = p_scl.tile([128, 16, 4], F32)   # SCALE * rstd_k per (sj_blk, kv)
            ctm = [p_ctm.tile([128, 1024], BF16, tag='ctm', name='ctm') for _ in range(8)]

            for kvh in range(4):  # ones column for the softmax denominator
                nc.gpsimd.memset(VA[kvh][:, :, 128:129], 1.0)

            bounds = [max(MYBLKS[0][bi], MYBLKS[1][bi]) for bi in range(8)]
            # per-core diagonal masks: dmask[bi][j] for j in {bounds[bi]-1, bounds[bi]}
            # encoded via a single input: dm [8, 2, 128, 128]
            dm = nc.dram_tensor('dm', [8, 2, 128, 128], F32, kind='ExternalInput')
            dm_s = p_c.tile([128, 8, 2, 128], BF16)
            nc.gpsimd.dma_start(dm_s[:], dm.rearrange('n t p d -> p n t d'))

            # ---- projections, two passes over s-halves ----
            for ph in range(2):
                ht_t = [p_ht.tile([128, 1024], BF16, tag='ht', name='ht') for _ in range(16)]
                for ch in range(16):
                    nc.gpsimd.dma_start(
                        ht_t[ch][:], hT[bass.ts(ch, 128), bass.ts(ph, 1024)])
                for j in range(8):
                    sb = ph * 8 + j
                    sslice = bass.ts(j, 128)
                    # ---- V ----
                    psV = ps_a.tile([128, 512], F32, tag='psA', name='psA')
                    for ch in range(16):
                        nc.tensor.matmul(psV[:], ht_t[ch][:, sslice], wv_s[ch][:],
                                         start=(ch == 0), stop=(ch == 15))
                    for kvh in range(4):
                        nc.scalar.copy(VA[kvh][:, sb, 0:128], psV[:, bass.ts(kvh, 128)])
                    # ---- K ----
                    psK = ps_a.tile([128, 512], F32, tag='psA', name='psA')
                    for ch in range(16):
                        nc.tensor.matmul(psK[:], ht_t[ch][:, sslice], wk_s[ch][:],
                                         start=(ch == 0), stop=(ch == 15))
                    kcp = p_w.tile([128, 512], F32, tag='kcp', name='kcp')
                    nc.scalar.copy(kcp[:], psK[:])
                    scr = p_w.tile([128, 512], F32, tag='scr', name='scr')
                    ss = p_s.tile([128, 4], F32, tag='ss', name='ss')
                    for kvh in range(4):
                        nc.scalar.activation(scr[:, bass.ts(kvh, 128)],
                                             kcp[:, bass.ts(kvh, 128)], AF.Square,
                                             accum_out=ss[:, kvh:kvh + 1])
                    nc.vector.tensor_scalar_add(ss[:], ss[:], float(EPS * D))
                    std = p_s.tile([128, 4], F32, tag='std', name='std')
                    nc.scalar.activation(std[:], ss[:], AF.Sqrt, scale=1.0 / D, bias=0.0)
                    rstd = p_s.tile([128, 4], F32, tag='rstd', name='rstd')
                    nc.vector.reciprocal(rstd[:], std[:])
                    nc.vector.tensor_scalar_mul(sclK[:, sb, :], rstd[:], SCALE)
                    # rope on raw K (w folded into ktab; rstd folded into exp scale)
                    kro = p_w.tile([128, 4, 128], BF16, tag='kro', name='kro')
                    lo = kcp[:].rearrange('p (t d) -> p t d', t=4)[:, :, 0:64]
                    hi = kcp[:].rearrange('p (t d) -> p t d', t=4)[:, :, 64:128]
                    tA = ktab_s[:, sb, :, :][:, 0:1, :]
                    tB = ktab_s[:, sb, :, :][:, 1:2, :]
                    tC = ktab_s[:, sb, :, :][:, 2:3, :]
                    tD = ktab_s[:, sb, :, :][:, 3:4, :]
                    t1 = p_w.tile([128, 4, 64], F32, tag='t1', name='t1')
                    t2 = p_w.tile([128, 4, 64], F32, tag='t2', name='t2')
                    mul_b(nc, t1[:], lo, tA)
                    mul_b(nc, t2[:], hi, tB)
                    nc.vector.tensor_sub(kro[:, :, 0:64], t1[:], t2[:])
                    mul_b(nc, t1[:], hi, tC)
                    mul_b(nc, t2[:], lo, tD)
                    nc.vector.tensor_add(kro[:, :, 64:128], t1[:], t2[:])
                    for kvh in range(4):  # transpose to KT
                        pst = ps_t.tile([128, 128], BF16, tag='psT', name='psT')
                        nc.tensor.transpose(pst[:], kro[:, kvh, :], iden_s[:])
                        nc.scalar.copy(KT[kvh][:, bass.ts(sb, 128)], pst[:])
                del ht_t

            # ---- Q projection from host-gathered hTq (my si rows, local order) ----
            hTq = nc.dram_tensor('hTq', [HID, 1024], F32, kind='ExternalInput')
            htq_t = [p_ht.tile([128, 1024], BF16, tag='ht', name='ht') for _ in range(16)]
            for ch in range(16):
                nc.gpsimd.dma_start(htq_t[ch][:], hTq[bass.ts(ch, 128), :])
            for bi in range(8):
                sslice = bass.ts(bi, 128)
                for qg in range(2):
                    psQ = ps_a.tile([128, 512], F32, tag='psA', name='psA')
                    for ch in range(16):
                        nc.tensor.matmul(psQ[:], htq_t[ch][:, sslice],
                                         wq_s[ch][:, bass.ts(qg, 512)],
                                         start=(ch == 0), stop=(ch == 15))
                    qcp = p_w.tile([128, 512], F32, tag='kcp', name='qcp')
                    nc.scalar.copy(qcp[:], psQ[:])
                    scr = p_w.tile([128, 512], F32, tag='scr', name='scr')
                    ss = p_s.tile([128, 4], F32, tag='ss', name='ss')
                    for hq in range(4):
                        nc.scalar.activation(scr[:, bass.ts(hq, 128)],
                                             qcp[:, bass.ts(hq, 128)], AF.Square,
                                             accum_out=ss[:, hq:hq + 1])
                    nc.vector.tensor_scalar_add(ss[:], ss[:], float(EPS * D))
                    std = p_s.tile([128, 4], F32, tag='std', name='std')
                    nc.scalar.activation(std[:], ss[:], AF.Sqrt, scale=1.0 / D, bias=0.0)
                    rstd = p_s.tile([128, 4], F32, tag='rstd', name='rstd')
                    nc.vector.reciprocal(rstd[:], std[:])
                    qro = p_w.tile([128, 4, 128], BF16, tag='kro', name='kro')
                    lo = qcp[:].rearrange('p (t d) -> p t d', t=4)[:, :, 0:64]
                    hi = qcp[:].rearrange('p (t d) -> p t d', t=4)[:, :, 64:128]
                    tA = qtab_s[:, bi, :, :][:, 0:1, :]
                    tB = qtab_s[:, bi, :, :][:, 1:2, :]
                    tC = qtab_s[:, bi, :, :][:, 2:3, :]
                    tD = qtab_s[:, bi, :, :][:, 3:4, :]
                    t1 = p_w.tile([128, 4, 64], F32, tag='t1', name='t1')
                    t2 = p_w.tile([128, 4, 64], F32, tag='t2', name='t2')
                    mul_b(nc, t1[:], lo, tA)
                    mul_b(nc, t2[:], hi, tB)
                    nc.vector.tensor_sub(qro[:, :, 0:64], t1[:], t2[:])
                    mul_b(nc, t1[:], hi, tC)
                    mul_b(nc, t2[:], lo, tD)
                    nc.vector.tensor_add(qro[:, :, 64:128], t1[:], t2[:])
                    qn = p_w.tile([128, 4, 128], BF16, tag='qn', name='qn')
                    for hq in range(4):
                        nc.vector.tensor_scalar_mul(qn[:, hq, :], qro[:, hq, :],
                                                    rstd[:, hq:hq + 1])
                        pst = ps_t.tile([128, 128], BF16, tag='psT', name='psT')
                        nc.tensor.transpose(pst[:], qn[:, hq, :], iden_s[:])
                        nc.scalar.copy(QT[qg * 4 + hq][:, bass.ts(bi, 128)], pst[:])

            # ---- attention ----
            for h in range(8):
                kvh = h // 2
                for bi in range(8):
                    gi = bounds[bi]
                    psC = ps_c.tile([128, 132], F32, tag='psC', name='psC')
                    for j in range(gi + 1):
                        psS = ps_s.tile([128, 128], F32, tag='psS', name='psS')
                        nc.tensor.matmul(psS[:], KT[kvh][:, bass.ts(j, 128)],
                                         QT[h][:, bass.ts(bi, 128)],
                                         start=True, stop=True)
                        ex = p_exp.tile([128, 128], BF16, tag='expb', name='expb')
                        nc.scalar.activation(ex[:], psS[:], AF.Exp,
                                             scale=sclK[:, j, kvh:kvh + 1])
                        if j >= gi - 1:  # possible diagonal/overhang: apply mask
                            nc.vector.tensor_mul(ex[:], ex[:], dm_s[:, bi, j - (gi - 1), :])
                        nc.tensor.matmul(psC[:, 0:129], ex[:], VA[kvh][:, j, 0:129],
                                         start=(j == 0), stop=(j == gi))
                    rd = p_s.tile([128, 1], F32, tag='rd', name='rd')
                    nc.vector.reciprocal(rd[:], psC[:, 128:129])
                    cn = p_w.tile([128, 128], BF16, tag='cn', name='cn')
                    nc.vector.tensor_scalar_mul(cn[:], psC[:, 0:128], rd[:])
                    pst = ps_t.tile([128, 128], BF16, tag='psT', name='psT')
                    nc.tensor.transpose(pst[:], cn[:], iden_s[:])
                    nc.scalar.copy(ctm[h][:, bass.ts(bi, 128)], pst[:])

            wo_s = [p_ht.tile([128, 1024], BF16, tag='ht', name='wo') for _ in range(16)]
            for ch in range(16):
                nc.gpsimd.dma_start(wo_s[ch][:], owT[bass.ts(ch, 128), :])

            # ---- pair AllGather of ctx^T ----
            cc_in = p_d.tile([1024, 1024], BF16)
            cc_out = p_d.tile([2048, 1024], BF16)
            for h in range(8):
                nc.sync.dma_start(cc_in[bass.ts(h, 128), :], ctm[h][:])
            nc.gpsimd.collective_compute(
                'AllGather', mybir.AluOpType.bypass,
                replica_groups=[[0, 1], [2, 3], [4, 5], [6, 7]],
                ins=[cc_in.opt()], outs=[cc_out.opt()])
            ctf = [p_wq.tile([128, 1024], BF16, tag='wq', name='ctf') for _ in range(16)]
            for ch in range(16):
                nc.sync.dma_start(ctf[ch][:], cc_out[bass.ts(ch, 128), :])

            # ---- o_proj (my ho half columns) ----
            for bi in range(8):
                for nt in range(2):
                    psO = ps_a.tile([128, 512], F32, tag='psA', name='psA')
                    for ch in range(16):
                        nc.tensor.matmul(psO[:], ctf[ch][:, bass.ts(bi, 128)],
                                         wo_s[ch][:, bass.ts(nt, 512)],
                                         start=(ch == 0), stop=(ch == 15))
                    ob = p_out.tile([128, 512], F32, tag='outb', name='outb')
                    nc.scalar.copy(ob[:], psO[:])
                    nc.sync.dma_start(out_e[bass.ts(bi, 128), bass.ts(nt, 512)], ob[:])

    split_multi_waits(nc)
    return nc


def mul_b(nc, out, a, b):
    """tensor_tensor multiply with free-dim broadcast of b over dim 1."""
    a2, b2 = bass.broadcast_tensor_aps(a, b)
    nc.vector.tensor_mul(out, a2, b2)


# ---------------------------------------------------------------------------
_NC_CACHE = None


def _get_nc():
    global _NC_CACHE
    if _NC_CACHE is None:
        _NC_CACHE = build_kernel()
    return _NC_CACHE


def kernel(hidden_states, cos, sin, q_w, k_w, v_w, o_w, q_norm_w, k_norm_w):
    from concourse.bass_utils import run_bass_kernel_spmd

    hidden_states = np.asarray(hidden_states, np.float32)
    cos = np.asarray(cos, np.float32)
    sin = np.asarray(sin, np.float32)
    q_w = np.asarray(q_w, np.float32)
    k_w = np.asarray(k_w, np.float32)
    v_w = np.asarray(v_w, np.float32)
    o_w = np.asarray(o_w, np.float32)
    q_norm_w = np.asarray(q_norm_w, np.float32)
    k_norm_w = np.asarray(k_norm_w, np.float32)

    tri_np = np.triu(np.ones((128, 128), np.float32))  # [sj,si]: valid sj<=si
    iden_np = np.eye(128, dtype=np.float32)

    def rope_tabs(c, s_, w):
        # tables [rows, 4, 64]: A=c_lo*w_lo, B=s_lo*w_hi, C=c_lo*w_hi, D=s_lo*w_lo
        cl, sl = c[:, 0:64], s_[:, 0:64]
        wl, wh = w[0:64], w[64:128]
        return np.stack([cl * wl, sl * wh, cl * wh, sl * wl], axis=1).astype(np.float32)

    bounds = [max(MYBLKS[0][bi], MYBLKS[1][bi]) for bi in range(8)]

    in_maps = []
    for c in range(8):
        b, sh, hh = c >> 2, (c >> 1) & 1, c & 1
        blks = MYBLKS[sh]
        rows = np.concatenate([np.arange(g * 128, (g + 1) * 128) for g in blks])
        hT = np.ascontiguousarray(hidden_states[b].T)
        hTq = np.ascontiguousarray(hidden_states[b][rows].T)
        qwT = np.ascontiguousarray(q_w[hh * 1024:(hh + 1) * 1024].T)
        kwT = np.ascontiguousarray(k_w[hh * 512:(hh + 1) * 512].T)
        vwT = np.ascontiguousarray(v_w[hh * 512:(hh + 1) * 512].T)
        owT = np.ascontiguousarray(o_w[hh * 1024:(hh + 1) * 1024].T)
        qtab = rope_tabs(cos[b][rows], sin[b][rows], q_norm_w)
        ktab = rope_tabs(cos[b], sin[b], k_norm_w)
        # diagonal masks dm[bi, t]: t=0 -> sj block gi-1, t=1 -> sj block gi
        # my true causal diagonal is at block g=blks[bi] (<= bounds[bi]).
        dm = np.zeros((8, 2, 128, 128), np.float32)
        for bi in range(8):
            g, gb = blks[bi], bounds[bi]
            for t, j in enumerate((gb - 1, gb)):
                if j < 0:
                    continue
                if j < g:
                    dm[bi, t] = 1.0
                elif j == g:
                    dm[bi, t] = tri_np
                # j > g: stays 0 (block fully masked)
        in_maps.append(dict(
            hT=hT, hTq=hTq, qwT=qwT, kwT=kwT, vwT=vwT, owT=owT,
            qtab=qtab, ktab=ktab, tri=tri_np, iden=iden_np, dm=dm))

    global _LAST_IN_MAPS
    _LAST_IN_MAPS = in_maps
    nc = _get_nc()
    res = run_bass_kernel_spmd(nc, in_maps, core_ids=list(range(8)))

    out = np.zeros((B, S, HID), np.float32)
    for c in range(8):
        b, sh, hh = c >> 2, (c >> 1) & 1, c & 1
        o = res.results[c]['out']  # [1024, 1024]
        for bi, g in enumerate(MYBLKS[sh]):
            out[b, g * 128:(g + 1) * 128, hh * 1024:(hh + 1) * 1024] = \
                o[bi * 128:(bi + 1) * 128]
    return out


if __name__ == '__main__':
    sys.path.insert(0, '/root/problem')
    import reference
    inputs = {k: np.asarray(v) for k, v in reference.setup_inputs().items()}
    exp = np.asarray(reference.reference(**inputs))
    act = kernel(**inputs)
    err = np.abs(act - exp)
    rel = np.linalg.norm(act - exp) / np.linalg.norm(exp)
    print('Relative error:', rel, 'max abs err:', err.max())



# revision 14
# speedup vs baseline: 1.7292x; 1.7292x over previous
"""Trainium2 Bass kernel for Qwen-style GQA attention block (B=2,S=2048,H=16,KV=8,D=128).

Sharding (8 cores): batch(2) x si-stripes(2) x head-half(2).
  core c: b=c>>2, sh=(c>>1)&1, hh=c&1
  - Q proj + attention for 8 q-heads (hh half) on 8 causally-balanced si blocks
  - K/V proj for 4 kv heads over full S (replicated across the 2 stripe cores)
  - two pair AllGathers of ctx^T (kvh01 early, kvh23 late), then column-split
    o_proj in two passes so the second collective overlaps the first pass.
All inputs pre-cast to bf16 on host. Matmuls bf16 with fp32 PSUM.
Softmax without max-subtraction; denominator via an appended ones-column on V.
Attention: K-stationary wide-f QK per (kvh, si-half, j) over all valid si
blocks x 2 heads, one wide exp, PV accumulates ctx [si,129] per (h, bi) in
a single 4-bank PSUM tile.
"""
import sys

sys.path.insert(0, '/opt/trn_rl_repo')

import numpy as np

import concourse.bass as bass
import concourse.tile as tile
from concourse import mybir
from concourse.vector_clock import ScopedClock, VectorClock

B, S, HID = 2, 2048, 2048
H, KV, D = 16, 8, 128
EPS = 1e-6
SCALE = D ** -0.5
NBLK = S // 128  # 16
# causally balanced si-block stripes; bounds[bi] = 2*bi+1 = max over stripes
MYBLKS = [[0, 2, 4, 6, 9, 11, 13, 15], [1, 3, 5, 7, 8, 10, 12, 14]]
BOUNDS = [1, 3, 5, 7, 9, 11, 13, 15]

F32 = mybir.dt.float32
BF16 = mybir.dt.bfloat16
AF = mybir.ActivationFunctionType
ALU = mybir.AluOpType

import ml_dtypes
BF16_NP = ml_dtypes.bfloat16


# ---------------------------------------------------------------------------
# Workarounds: this walrus supports only ONE sync-wait per instruction.
def _patched_drain_and_barrier(self, tick_clock, wait_clock):
    gc = tick_clock.global_clock
    vec = list(gc)
    nz = [i for i, v in enumerate(vec) if v > 0] or [0]
    for i in nz:
        cvec = [vec[j] if j == i else 0 for j in range(len(vec))]
        inst = self.nc.sync.drain()
        wait_clock.add_sem_waits(inst.ins, ScopedClock({None: VectorClock(cvec)}))
    self.nc.all_engine_barrier()
    assert self.sems is not None
    popped = self.nc._tile_sem_poison_stack.pop()
    assert popped is self._sem_poison
    self.nc.clear_and_free_semaphores(list(self.sems.allocated().values()))
    self.nc.all_engine_barrier()


tile.TileContext._drain_and_barrier = _patched_drain_and_barrier


def split_multi_waits(nc):
    for fn in nc.m.functions:
        for blk in fn.blocks:
            insts = list(blk.instructions)
            out = []
            changed = False
            for inst in insts:
                si = inst.sync_info
                if si is not None and len(si.on_wait) > 1:
                    waits = list(si.on_wait)
                    for k, w in enumerate(waits[:-1]):
                        out.append(mybir.InstNoOp(
                            name=f"{inst.name}.w{k}", engine=inst.engine,
                            sync_info=mybir.SyncInfo(on_wait=[w], on_update=[]),
                            text_hint="waitsplit"))
                    si.on_wait = [waits[-1]]
                    changed = True
                out.append(inst)
            if changed:
                blk.instructions[:] = out


def mul_b(nc, out, a, b):
    """tensor_tensor multiply with free-dim broadcast of b."""
    a2, b2 = bass.broadcast_tensor_aps(a, b)
    nc.vector.tensor_mul(out, a2, b2)


# ---------------------------------------------------------------------------
DEBUG_DUMP = False


def build_kernel():
    nc = bass.Bass(trn_type='TRN2')
    hT = nc.dram_tensor('hT', [HID, S], BF16, kind='ExternalInput')
    hTq = nc.dram_tensor('hTq', [HID, 1024], BF16, kind='ExternalInput')
    wkvT = nc.dram_tensor('wkvT', [HID, 1024], BF16, kind='ExternalInput')
    qwT = nc.dram_tensor('qwT', [HID, 1024], BF16, kind='ExternalInput')
    owT = nc.dram_tensor('owT', [2048, 1024], BF16, kind='ExternalInput')
    # fused rope tables [rows, 128]: AC = [w_lo*cos | w_hi*cos],
    # DB = [w_lo*sin | w_hi*sin]; t1 = x*AC, t2 = x*DB;
    # out_lo = t1_lo - t2_hi, out_hi = t1_hi + t2_lo
    qtAC = nc.dram_tensor('qtAC', [1024, 128], BF16, kind='ExternalInput')
    qtDB = nc.dram_tensor('qtDB', [1024, 128], BF16, kind='ExternalInput')
    ktAC = nc.dram_tensor('ktAC', [S, 128], BF16, kind='ExternalInput')
    ktDB = nc.dram_tensor('ktDB', [S, 128], BF16, kind='ExternalInput')
    iden = nc.dram_tensor('iden', [128, 128], BF16, kind='ExternalInput')
    dm = nc.dram_tensor('dm', [2, 2, 128, 128], BF16, kind='ExternalInput')
    out_e = nc.dram_tensor('out', [1024, 1024], F32, kind='ExternalOutput')

    from contextlib import ExitStack
    with ExitStack() as octx:
        tc = octx.enter_context(tile.TileContext(nc))
        pool = lambda ctx, name, bufs, **kw: ctx.enter_context(
            tc.tile_pool(name=name, bufs=bufs, **kw))
        # ---- persistent pools ----
        p_c = pool(octx, 'const', 1)
        p_kt = pool(octx, 'ktq', 1)
        p_work = pool(octx, 'work', 2)
        p_s = pool(octx, 'small', 4)
        p_d = pool(octx, 'dram', 1, space='DRAM')

        iden_s = p_c.tile([128, 128], BF16)
        nc.gpsimd.dma_start(iden_s[:], iden[:])
        qtAC_s = p_c.tile([128, 8, 128], BF16)
        nc.gpsimd.dma_start(qtAC_s[:], qtAC.rearrange('(n p) d -> p n d', p=128))
        qtDB_s = p_c.tile([128, 8, 128], BF16)
        nc.gpsimd.dma_start(qtDB_s[:], qtDB.rearrange('(n p) d -> p n d', p=128))
        ktAC_s = p_c.tile([128, 16, 128], BF16)
        nc.gpsimd.dma_start(ktAC_s[:], ktAC.rearrange('(n p) d -> p n d', p=128))
        ktDB_s = p_c.tile([128, 16, 128], BF16)
        nc.gpsimd.dma_start(ktDB_s[:], ktDB.rearrange('(n p) d -> p n d', p=128))
        dm_s = p_c.tile([128, 2, 2, 128], BF16)
        nc.gpsimd.dma_start(dm_s[:], dm.rearrange('h t p d -> p h t d'))

        KT = p_kt.tile([128, 4, 16, 128], BF16)    # [d, kvh, sj_blk, sj]
        VA = p_kt.tile([128, 4, 16, 132], BF16)    # [sj, kvh, sj_blk, d|ones]
        QT = p_kt.tile([128, 8, 8, 128], BF16)     # [d, h, bi, si]
        sclK = p_kt.tile([128, 16, 4], F32)        # SCALE*rstd_k [sj,(blk,kvh)]
        nc.gpsimd.memset(VA[:, :, :, 128:129], 1.0)

        cc1_in = p_d.tile([512, 1024], BF16)
        cc1_out = p_d.tile([1024, 1024], BF16)
        cc2_in = p_d.tile([512, 1024], BF16)
        cc2_out = p_d.tile([1024, 1024], BF16)

        # ============ phase B: K/V proj, phase C: Q proj ============
        with ExitStack() as ctx1:
            p_w8 = pool(ctx1, 'wts', 16)
            p_ht = pool(ctx1, 'htp', 16)
            ps_kv = pool(ctx1, 'psKV', 2, space='PSUM')
            ps_t = pool(ctx1, 'psTt', 2, space='PSUM')

            wkv_s = [p_w8.tile([128, 1024], BF16, tag='w', name='wkv')
                     for _ in range(16)]
            for ch in range(16):
                nc.gpsimd.dma_start(wkv_s[ch][:], wkvT[bass.ts(ch, 128), :])
            wq_s = [p_w8.tile([128, 1024], BF16, tag='wq', name='wq')
                    for _ in range(16)]

            for ph in range(2):
                ht_t = [p_ht.tile([128, 1024], BF16, tag='ht', name='ht')
                        for _ in range(16)]
                for ch in range(16):
                    nc.gpsimd.dma_start(
                        ht_t[ch][:], hT[bass.ts(ch, 128), bass.ts(ph, 1024)])
                for jj in range(8):
                    sb = ph * 8 + jj
                    sslice = bass.ts(jj, 128)
                    psKV = ps_kv.tile([128, 1024], F32, tag='psKV', name='psKV')
                    for ch in range(16):
                        for hf in range(2):
                            nc.tensor.matmul(
                                psKV[:, bass.ts(hf, 512)],
                                ht_t[ch][:, sslice],
                                wkv_s[ch][:, bass.ts(hf, 512)],
                                start=(ch == 0), stop=(ch == 15))
                    kvcp = p_work.tile([128, 1024], F32, tag='big',
                                       name='kvcp')
                    nc.scalar.copy(kvcp[:], psKV[:])
                    # V -> VA (one strided gpsimd copy, SBUF->SBUF)
                    nc.gpsimd.tensor_copy(
                        VA[:, :, sb, 0:128],
                        kvcp[:, 512:1024].rearrange('p (k d) -> p k d', k=4))
                    kv = kvcp[:, 0:512].rearrange('p (k d) -> p k d', k=4)
                    scr = p_work.tile([128, 4, 128], F32, tag='scr', name='scr')
                    ss = p_s.tile([128, 4], F32, tag='ss', name='ss')
                    for k in range(4):
                        nc.scalar.activation(scr[:, k, :], kv[:, k, :],
                                             AF.Square,
                                             accum_out=ss[:, k:k + 1])
                    nc.vector.tensor_scalar_add(ss[:], ss[:], float(EPS * D))
                    std = p_s.tile([128, 4], F32, tag='std', name='std')
                    nc.scalar.activation(std[:], ss[:], AF.Sqrt,
                                         scale=1.0 / D, bias=0.0)
                    rstd = p_s.tile([128, 4], F32, tag='rstd', name='rstd')
                    nc.vector.reciprocal(rstd[:], std[:])
                    nc.vector.tensor_scalar_mul(sclK[:, sb, :], rstd[:], SCALE)
                    # K rope
                    t1 = p_work.tile([128, 4, 128], F32, tag='t1', name='t1')
                    t2 = p_work.tile([128, 4, 128], F32, tag='t2', name='t2')
                    mul_b(nc, t1[:], kv, ktAC_s[:, sb:sb + 1, :])
                    mul_b(nc, t2[:], kv, ktDB_s[:, sb:sb + 1, :])
                    kro = p_work.tile([128, 4, 128], BF16, tag='ro', name='kro')
                    nc.vector.tensor_sub(kro[:, :, 0:64], t1[:, :, 0:64],
                                         t2[:, :, 64:128])
                    nc.vector.tensor_add(kro[:, :, 64:128], t1[:, :, 64:128],
                                         t2[:, :, 0:64])
                    pst = ps_t.tile([128, 4, 128], BF16, tag='psT', name='psT')
                    for k in range(4):
                        nc.tensor.transpose(pst[:, k, :], kro[:, k, :], iden_s[:])
                    nc.scalar.copy(KT[:, :, sb, :], pst[:])
                del ht_t

            # ---- Q proj over my 1024 si rows (host-gathered local order) ----
            for ch in range(16):
                nc.gpsimd.dma_start(wq_s[ch][:], qwT[bass.ts(ch, 128), :])
            htq_t = [p_w8.tile([128, 1024], BF16, tag='w', name='htq')
                     for _ in range(16)]
            for ch in range(16):
                nc.gpsimd.dma_start(htq_t[ch][:], hTq[bass.ts(ch, 128), :])
            for bi in range(8):
                sslice = bass.ts(bi, 128)
                psQ = ps_kv.tile([128, 1024], F32, tag='psKV', name='psQ')
                for ch in range(16):
                    for hf in range(2):
                        nc.tensor.matmul(
                            psQ[:, bass.ts(hf, 512)],
                            htq_t[ch][:, sslice],
                            wq_s[ch][:, bass.ts(hf, 512)],
                            start=(ch == 0), stop=(ch == 15))
                qcp = p_work.tile([128, 1024], F32, tag='big', name='qcp')
                nc.scalar.copy(qcp[:], psQ[:])
                qv = qcp[:].rearrange('p (h d) -> p h d', h=8)
                scr = p_work.tile([128, 8, 128], F32, tag='scr', name='scr8')
                ss = p_s.tile([128, 8], F32, tag='ss8', name='ss8')
                for h in range(8):
                    nc.scalar.activation(scr[:, h, :], qv[:, h, :],
                                         AF.Square,
                                         accum_out=ss[:, h:h + 1])
                nc.vector.tensor_scalar_add(ss[:], ss[:], float(EPS * D))
                std = p_s.tile([128, 8], F32, tag='std8', name='std8')
                nc.scalar.activation(std[:], ss[:], AF.Sqrt,
                                     scale=1.0 / D, bias=0.0)
                rstd = p_s.tile([128, 8], F32, tag='rstd8', name='rstd8')
                nc.vector.reciprocal(rstd[:], std[:])
                # fold rstd into Q before rope (rope is per-head linear)
                qs = p_work.tile([128, 8, 128], F32, tag='big', name='qs')
                mul_b(nc, qs[:], qv,
                      rstd[:].rearrange('p (h o) -> p h o', o=1))
                t1 = p_work.tile([128, 8, 128], F32, tag='t1', name='t18')
                t2 = p_work.tile([128, 8, 128], F32, tag='t2', name='t28')
                mul_b(nc, t1[:], qs[:], qtAC_s[:, bi:bi + 1, :])
                mul_b(nc, t2[:], qs[:], qtDB_s[:, bi:bi + 1, :])
                qn = p_work.tile([128, 8, 128], BF16, tag='ro', name='qn')
                nc.vector.tensor_sub(qn[:, :, 0:64], t1[:, :, 0:64],
                                     t2[:, :, 64:128])
                nc.vector.tensor_add(qn[:, :, 64:128], t1[:, :, 64:128],
                                     t2[:, :, 0:64])
                pst = ps_t.tile([128, 8, 128], BF16, tag='psT', name='psT8')
                for h in range(8):
                    nc.tensor.transpose(pst[:, h, :], qn[:, h, :], iden_s[:])
                nc.scalar.copy(QT[:, :, bi, :], pst[:])

        ccd = None
        if DEBUG_DUMP:
            ccd = nc.dram_tensor('ccd', [4, 2, 128, 2, 4, 132], F32,
                                 kind='ExternalOutput')
            ktd = nc.dram_tensor('ktd', [128, 4, 16, 128], BF16,
                                 kind='ExternalOutput')
            vad = nc.dram_tensor('vad', [128, 4, 16, 132], BF16,
                                 kind='ExternalOutput')
            qtd = nc.dram_tensor('qtd', [128, 8, 8, 128], BF16,
                                 kind='ExternalOutput')
            scld = nc.dram_tensor('scld', [128, 16, 4], F32,
                                  kind='ExternalOutput')
            nc.sync.dma_start(ktd[:], KT[:])
            nc.sync.dma_start(vad[:], VA[:])
            nc.sync.dma_start(qtd[:], QT[:])
            nc.sync.dma_start(scld[:], sclK[:])

        # ============ phase D: attention, E/F: o_proj ============
        with ExitStack() as ctx2:
            p_wo = pool(ctx2, 'wop', 16)
            wo_s = [p_wo.tile([128, 1024], BF16, tag='wo', name='wo')
                    for _ in range(16)]
            for ch in range(16):
                nc.gpsimd.dma_start(wo_s[ch][:], owT[bass.ts(ch, 128), :])

            with ExitStack() as ctx2a:
                p_ex = pool(ctx2a, 'exp', 3)
                ps_s = pool(ctx2a, 'psSp', 2, space='PSUM')
                ps_c = pool(ctx2a, 'psCp', 1, space='PSUM')

                for kvh in range(4):
                    for half in range(2):
                        bis = list(range(half * 4, half * 4 + 4))
                        jmax = BOUNDS[bis[-1]]
                        js = list(range(jmax + 1))
                        lo = [sum(1 for b in bis if BOUNDS[b] < j) for j in js]
                        psS = {}
                        ex = {}

                        def emit_qk(j):
                            l = lo[j]
                            t = ps_s.tile([128, 2, 4, 128], F32, tag='psS',
                                          name='psS')
                            psS[j] = t
                            for h in range(2):
                                nc.tensor.matmul(
                                    t[:, h, l:4, :],
                                    KT[:, kvh, j, :],
                                    QT[:, 2 * kvh + h,
                                       half * 4 + l:half * 4 + 4, :],
                                    start=True, stop=True)

                        def emit_exp(j):
                            l = lo[j]
                            e = p_ex.tile([128, 2, 4, 128], BF16, tag='ex',
                                          name='ex')
                            ex[j] = e
                            nc.scalar.activation(
                                e[:, :, l:4, :], psS[j][:, :, l:4, :], AF.Exp,
                                scale=sclK[:, j, kvh:kvh + 1])
                            del psS[j]
                            # mask the single block with BOUNDS in {j, j+1}
                            slot = j // 2 - half * 4
                            if 0 <= slot < 4:
                                par = j % 2
                                mul_b(nc, e[:, :, slot, :], e[:, :, slot, :],
                                      dm_s[:, half:half + 1, par, :])

                        psC = ps_c.tile([128, 2, 4, 256], F32, tag='psC',
                                        name='psC')
                        emit_qk(0)
                        emit_exp(0)
                        for j in js:
                            if j + 1 <= jmax:
                                emit_qk(j + 1)
                                emit_exp(j + 1)
                            for h in range(2):
                                for slot in range(lo[j], 4):
                                    bi = half * 4 + slot
                                    # start=True clears has_written for the
                                    # WHOLE bank: issue it only for the first
                                    # region of each bank (even slot); the odd
                                    # slot's first write lands on cleared bits
                                    # and overwrites, then accumulates.
                                    nc.tensor.matmul(
                                        psC[:, h, slot, 0:129],
                                        ex[j][:, h, slot, :],
                                        VA[:, kvh, j, 0:129],
                                        start=(j == 0 and slot % 2 == 0),
                                        stop=(j == BOUNDS[bi]),
                                        skip_group_check=True)
                            if j - 1 in ex:
                                del ex[j - 1]
                        # epilogue: normalize, transpose, stage for allgather
                        ccp = p_ex.tile([128, 2, 4, 132], F32, tag='ccp',
                                        name='ccp', bufs=2)
                        nc.scalar.copy(ccp[:, :, :, 0:129],
                                       psC[:, :, :, 0:129])
                        rd = p_s.tile([128, 2, 4, 1], F32, tag='rd', name='rd')
                        nc.vector.reciprocal(rd[:], ccp[:, :, :, 128:129])
                        cn = p_ex.tile([128, 2, 4, 128], BF16, tag='cn',
                                       name='cn', bufs=2)
                        mul_b(nc, cn[:], ccp[:, :, :, 0:128], rd[:])
                        pst = ps_s.tile([128, 2, 4, 128], BF16, tag='psS',
                                        name='pstc')
                        for h in range(2):
                            for slot in range(4):
                                nc.tensor.transpose(pst[:, h, slot, :],
                                                    cn[:, h, slot, :],
                                                    iden_s[:])
                        if DEBUG_DUMP:
                            nc.sync.dma_start(ccd[kvh, half], ccp[:])
                        ctm = p_ex.tile([128, 2, 4, 128], BF16, tag='ctm',
                                        name='ctm', bufs=2)
                        nc.scalar.copy(ctm[:], pst[:])
                        cc_in = cc1_in if kvh < 2 else cc2_in
                        dst = cc_in.rearrange('(h d) (bi si) -> d h bi si',
                                              h=4, bi=8)
                        hh0 = (kvh % 2) * 2
                        nc.sync.dma_start(
                            dst[:, hh0:hh0 + 2, half * 4:half * 4 + 4, :],
                            ctm[:])
                    if kvh == 1:
                        nc.gpsimd.collective_compute(
                            'AllGather', mybir.AluOpType.bypass,
                            replica_groups=[[0, 1], [2, 3], [4, 5], [6, 7]],
                            ins=[cc1_in.opt()], outs=[cc1_out.opt()])
                nc.gpsimd.collective_compute(
                    'AllGather', mybir.AluOpType.bypass,
                    replica_groups=[[0, 1], [2, 3], [4, 5], [6, 7]],
                    ins=[cc2_in.opt()], outs=[cc2_out.opt()])

            # ---- o_proj: open-PSUM accumulation, two waves of 4 si
            # blocks; wave0 pass1 (kvh01 chunks) overlaps the second CC ----
            with ExitStack() as ctx2b:
                p_cf = pool(ctx2b, 'ctfp', 8)
                ps_o = pool(ctx2b, 'psOp', 4, space='PSUM')
                ctf1 = [p_cf.tile([128, 1024], BF16, tag='cf1', name='ctf1')
                        for _ in range(8)]
                for t in range(8):
                    nc.sync.dma_start(ctf1[t][:], cc1_out[bass.ts(t, 128), :])
                ctf2 = [p_cf.tile([128, 1024], BF16, tag='cf2', name='ctf2')
                        for _ in range(8)]
                for t in range(8):
                    nc.sync.dma_start(ctf2[t][:], cc2_out[bass.ts(t, 128), :])
                for wave in range(2):
                    psOs = {}
                    for bi in range(wave * 4, wave * 4 + 4):
                        psO = ps_o.tile([128, 1024], F32, tag='psO',
                                        name='psO')
                        psOs[bi] = psO
                        for t in range(8):
                            p, h = t // 4, t % 4
                            ch = p * 8 + h
                            for hf in range(2):
                                nc.tensor.matmul(
                                    psO[:, bass.ts(hf, 512)],
                                    ctf1[t][:, bass.ts(bi, 128)],
                                    wo_s[ch][:, bass.ts(hf, 512)],
                                    start=(t == 0), stop=False,
                                    skip_group_check=True)
                    for bi in range(wave * 4, wave * 4 + 4):
                        psO = psOs[bi]
                        for t in range(8):
                            p, h = t // 4, t % 4
                            ch = p * 8 + 4 + h
                            for hf in range(2):
                                nc.tensor.matmul(
                                    psO[:, bass.ts(hf, 512)],
                                    ctf2[t][:, bass.ts(bi, 128)],
                                    wo_s[ch][:, bass.ts(hf, 512)],
                                    start=False, stop=(t == 7),
                                    skip_group_check=True)
                        ob = p_work.tile([128, 1024], F32, tag='big',
                                         name='ob')
                        nc.scalar.copy(ob[:], psO[:])
                        nc.sync.dma_start(out_e[bass.ts(bi, 128), :], ob[:])

    split_multi_waits(nc)
    return nc


# ---------------------------------------------------------------------------
_NC_CACHE = None
_LAST_IN_MAPS = None
_LAST_RES = None


_NC_DUMP = None


def _get_nc():
    global _NC_CACHE, _NC_DUMP
    if _NC_CACHE is None or _NC_DUMP != DEBUG_DUMP:
        _NC_CACHE = build_kernel()
        _NC_DUMP = DEBUG_DUMP
    return _NC_CACHE


def kernel(hidden_states, cos, sin, q_w, k_w, v_w, o_w, q_norm_w, k_norm_w):
    global _LAST_IN_MAPS
    from concourse.bass_utils import run_bass_kernel_spmd

    hidden_states = np.asarray(hidden_states, np.float32)
    cos = np.asarray(cos, np.float32)
    sin = np.asarray(sin, np.float32)
    q_w = np.asarray(q_w, np.float32)
    k_w = np.asarray(k_w, np.float32)
    v_w = np.asarray(v_w, np.float32)
    o_w = np.asarray(o_w, np.float32)
    q_norm_w = np.asarray(q_norm_w, np.float32)
    k_norm_w = np.asarray(k_norm_w, np.float32)

    tri_np = np.triu(np.ones((128, 128), np.float32))  # [sj,si]: valid sj<=si
    iden_np = np.eye(128, dtype=np.float32)

    def rope_tabs(c, s_, w):
        cl, sl = c[:, 0:64], s_[:, 0:64]
        wl, wh = w[0:64], w[64:128]
        AC = np.concatenate([cl * wl, cl * wh], axis=1)
        DB = np.concatenate([sl * wl, sl * wh], axis=1)
        return AC.astype(BF16_NP), DB.astype(BF16_NP)

    in_maps = []
    for c in range(8):
        b, sh, hh = c >> 2, (c >> 1) & 1, c & 1
        blks = MYBLKS[sh]
        rows = np.concatenate([np.arange(g * 128, (g + 1) * 128) for g in blks])
        hT = np.ascontiguousarray(hidden_states[b].T).astype(BF16_NP)
        hTq = np.ascontiguousarray(hidden_states[b][rows].T).astype(BF16_NP)
        qwT = np.ascontiguousarray(
            q_w[hh * 1024:(hh + 1) * 1024].T).astype(BF16_NP)
        wkvT = np.ascontiguousarray(np.concatenate(
            [k_w[hh * 512:(hh + 1) * 512].T,
             v_w[hh * 512:(hh + 1) * 512].T], axis=1)).astype(BF16_NP)
        owT = np.ascontiguousarray(
            o_w[hh * 1024:(hh + 1) * 1024].T).astype(BF16_NP)
        qtAC, qtDB = rope_tabs(cos[b][rows], sin[b][rows], q_norm_w)
        ktAC, ktDB = rope_tabs(cos[b], sin[b], k_norm_w)
        # dm[half][parity] applied to block bi* = j//2 (BOUNDS in {j, j+1}):
        #   j even (== bound-1): light stripe (myblk==j): TRI, heavy: ONES
        #   j odd  (== bound):   light (myblk<j): ZERO,    heavy: TRI
        # stripe0 is light in half0 (blocks 0,2,4,6), heavy in half1
        dm = np.zeros((2, 2, 128, 128), np.float32)
        for half in range(2):
            light = (sh == 0) if half == 0 else (sh == 1)
            if light:
                dm[half, 0] = tri_np
                dm[half, 1] = 0.0
            else:
                dm[half, 0] = 1.0
                dm[half, 1] = tri_np
        in_maps.append(dict(
            hT=hT, hTq=hTq, wkvT=wkvT, qwT=qwT, owT=owT,
            qtAC=qtAC, qtDB=qtDB, ktAC=ktAC, ktDB=ktDB,
            iden=iden_np.astype(BF16_NP), dm=dm.astype(BF16_NP)))

    _LAST_IN_MAPS = in_maps
    nc = _get_nc()
    res = run_bass_kernel_spmd(nc, in_maps, core_ids=list(range(8)))
    global _LAST_RES
    _LAST_RES = res

    out = np.zeros((B, S, HID), np.float32)
    for c in range(8):
        b, sh, hh = c >> 2, (c >> 1) & 1, c & 1
        o = res.results[c]['out']  # [1024, 1024]
        for bi, g in enumerate(MYBLKS[sh]):
            out[b, g * 128:(g + 1) * 128, hh * 1024:(hh + 1) * 1024] = \
                o[bi * 128:(bi + 1) * 128]
    return out


if __name__ == '__main__':
    sys.path.insert(0, '/root/problem')
    import reference
    inputs = {k: np.asarray(v) for k, v in reference.setup_inputs().items()}
    exp = np.asarray(reference.reference(**inputs))
    act = kernel(**inputs)
    err = np.abs(act - exp)
    rel = np.linalg.norm(act - exp) / np.linalg.norm(exp)
    print('Relative error:', rel, 'max abs err:', err.max())
